# revision 19
# baseline (speedup 1.0000x reference)
"""Multi-head attention (B=2, S=2048, E=1024, H=16, hd=64) on 8 trn2 cores.

Sharding: core c handles batch b = c//4 and 4 heads h0 = 4*(c%4).
Each core computes its heads' attention output projected through its rows
of Wo (tensor-parallel row split); the host sums the 4 partials per batch
and adds bo.

Device-side dataflow (per core, feature-major "transposed" layouts):
  xT   [e,s]  bf16  <- host-pretransposed x[b]
  qT,kT[d,s]  f32   <- Wqkv_slice^T @ xT   (bf16 matmul, fp32 psum)
  v1   [s,d]  bf16  <- xT-tile-stationary matmul (natural v, no transpose)
  scT  [k,2*q] f32  <- kT^T @ qT, both heads of a pair in one psum tile
  ex   [k,2*q] bf16 <- exp(scT/8) (one Act op per k-tile), diag tri mask
  ops  [65,2*q] f32 <- [v|1]^T @ ex  (bf16 matmul, fused denominator row)
  oT   [d,q]  f32r  <- ops * (1/den) (Pool partition_broadcast + DVE/Pool mul)
  y    [q,e]  bf16  <- oT^T @ Wo_slice (f32r matmul), psum->sbuf->HBM
"""

import os
import sys

sys.path.insert(0, "/opt/trn_rl_repo")

from contextlib import ExitStack

import ml_dtypes
import numpy as np

import concourse.bass as bass
import concourse.tile as tile
from concourse import bacc, mybir
from concourse._compat import with_exitstack
from concourse.bass_utils import run_bass_kernel_spmd

B, S, E, H = 2, 2048, 1024, 16
HD = E // H            # 64
NH = 4                 # heads per core
HC = NH * 3 * HD       # 768 qkv columns per core
ET = E // 128          # 8 e-tiles
QSB = S // 512         # 4 query super-blocks
KT = S // 128          # 16 k tiles
VP = 80                # v1 per-head stride: 64 v cols + 1 ones col + pad
F32 = mybir.dt.float32
F32R = mybir.dt.float32r
BF16 = mybir.dt.bfloat16

_CACHE = {}
LAST_RESULT = None

ALL_PHASES = frozenset({"x", "qkv", "v", "sc", "av", "wo"})


@with_exitstack
def _mha_kernel(ctx: ExitStack, tc: tile.TileContext, x, wqkv, b2, wo, vbias,
                tri, yp, phases=ALL_PHASES):
    nc = tc.nc

    const = ctx.enter_context(tc.tile_pool(name="const", bufs=1))
    work = ctx.enter_context(tc.tile_pool(name="work", bufs=1))
    psum = ctx.enter_context(tc.tile_pool(name="psum", bufs=1, space="PSUM"))

    # ---- persistent SBUF tensors ----
    W = const.tile([128, ET, HC], BF16)          # Wqkv slice, e-major
    Wor = const.tile([128, 2, E], BF16)          # Wo slice rows (hd), bf16
    bqk = const.tile([128, 6], F32)              # q/k bias, d-major per m-tile
    vb_t = const.tile([128, NH * HD], F32)       # v bias broadcast over rows
    trit = const.tile([128, 2, 128], BF16)       # causal tri mask, both heads
    xT = const.tile([128, ET, S], BF16)          # x[b] transposed, bf16
    qT = const.tile([128, 2, S], BF16)           # per head-pair: even@0:64 rows
    kT = const.tile([128, 2, S], BF16)
    v1 = const.tile([128, KT, NH * VP], BF16)    # v natural + ones col per head

    # ---- const loads (sync queue; small tensors first, they gate compute)
    nc.sync.dma_start(bqk[:], b2[:, :])
    nc.sync.dma_start(vb_t[:], vbias[:, :])
    nc.sync.dma_start(trit[:], tri[:, :].bitcast(BF16).rearrange(
        "p (h q) -> p h q", h=2))
    # W as 3 column-slab DMAs (q|k|v), one issue each, 16-engine parallel
    wq3 = wqkv[:, :].bitcast(BF16).rearrange("(t p) c -> p t c", p=128)
    for c0, c1 in ((0, 256), (256, 512), (512, 768)):
        nc.sync.dma_start(W[:, :, c0:c1], wq3[:, :, c0:c1])
    # x as 4 s-block slabs: qkv(n) gated only on slab n
    for n in range(QSB) if "x" in phases else []:
        nc.sync.dma_start(
            xT[:, :, n * 512:(n + 1) * 512],
            x[:, :].bitcast(BF16).rearrange(
                "(t p) s -> p t s", p=128)[:, :, n * 512:(n + 1) * 512])
    nc.sync.dma_start(
        Wor[:],
        wo[:, :].bitcast(BF16).rearrange("(t p) c -> p t c", p=128))
    for h in range(NH):
        nc.vector.memset(v1[:, :, h * VP + HD:h * VP + HD + 1], 1.0)
    ones64 = const.tile([1, 64], BF16, name="ones64")
    nc.vector.memset(ones64[:], 1.0)
    # warm the Act table (Exp) while DMAs stream, off the critical path
    warm = work.tile([1, 64], F32, name="warm", tag="warm", bufs=1)
    nc.scalar.activation(warm[:], ones64[:],
                         mybir.ActivationFunctionType.Exp, scale=0.125)

    oT_hist = {}

    def qkv_qk(n, fin_pend):
        # qT/kT for s-block n: W-tile stationary, xT moving
        for m in range(4):
            ps = psum.tile([128, 512], F32, tag="acc", bufs=2)
            for et in range(ET):
                nc.tensor.matmul(
                    ps[:],
                    W[:, et, m * 128:(m + 1) * 128],
                    xT[:, et, n * 512:(n + 1) * 512],
                    start=(et == 0),
                    stop=(et == ET - 1),
                )
            typ, hp = m // 2, m % 2
            dst = (qT if typ == 0 else kT)[:, hp, n * 512:(n + 1) * 512]
            if typ == 0:
                nc.vector.tensor_scalar_add(dst, ps[:], bqk[:, m:m + 1])
            else:
                nc.scalar.activation(
                    dst, ps[:], mybir.ActivationFunctionType.Identity,
                    bias=bqk[:, m:m + 1])
            if m == 0 and fin_pend:
                fin_pend.pop(0)()

    def v_nat(n):
        # natural-layout v via xT-tile-stationary matmul (no PE transpose)
        for st in range(4 * n, 4 * n + 4):
            vp = psum.tile([128, NH * HD], F32, name="vp", tag="big", bufs=2)
            for et in range(ET):
                nc.tensor.matmul(
                    vp[:],
                    xT[:, et, st * 128:(st + 1) * 128],
                    W[:, et, 512:768],
                    start=(et == 0),
                    stop=(et == ET - 1),
                )
            dst = v1[:, st, :].rearrange("p (h c) -> p h c", h=NH)[:, :, :HD]
            nc.vector.tensor_add(
                dst,
                vp[:].rearrange("p (h c) -> p h c", h=NH),
                vb_t[:].rearrange("p (h c) -> p h c", h=NH),
            )

    def attn(n, fin_pend):
        nkt = 4 * (n + 1)
        oT = [
            work.tile([128, 512], BF16, name=f"oT{hp}", tag=f"oT{hp}", bufs=2)
            for hp in range(2)
        ]
        oT_hist[n] = oT

        def make_fin(hp, ops):
            def fin():
                rc = work.tile([1, 1024], BF16, tag="rc", bufs=2)
                with nc.allow_low_precision(reason="den fits bf16"):
                    nc.vector.reciprocal(rc[:], ops[64:65, :])
                rb = psum.tile([64, 1024], F32, name="rb", tag="big", bufs=2)
                for h2 in range(2):
                    nc.tensor.matmul(
                        rb[:, h2 * 512:(h2 + 1) * 512],
                        ones64[:],
                        rc[:, h2 * 512:(h2 + 1) * 512],
                        start=True, stop=True)
                rbs = work.tile([64, 1024], F32, tag="rbs", bufs=2)
                nc.vector.tensor_copy(rbs[:], rb[:])
                for h2 in range(2):
                    nc.vector.tensor_mul(
                        oT[hp][64 * h2:64 * h2 + 64, :],
                        ops[0:64, h2 * 512:(h2 + 1) * 512],
                        rbs[0:64, h2 * 512:(h2 + 1) * 512])
            return fin

        pend = None
        for hp in range(2):
            ops = psum.tile([65, 1024], F32, name="ops", tag="acc", bufs=2)

            def emit_av(kt, q0, ex, ops=ops, hp=hp):
                for h2 in range(2):
                    h = 2 * hp + h2
                    nc.tensor.matmul(
                        ops[:, h2 * 512 + q0:(h2 + 1) * 512],
                        v1[:, kt, h * VP:h * VP + HD + 1],
                        ex[:, h2, q0:],
                        start=(kt == 0),
                        stop=(kt == nkt - 1),
                    )

            for kt in range(nkt):
                j = kt - 4 * n
                q0 = max(0, j * 128)
                sc = psum.tile([128, 1024], F32, tag="big", bufs=2)
                if "sc" in phases:
                    for h2 in range(2):
                        b0 = 64 * h2
                        nc.tensor.matmul(
                            sc[:, h2 * 512 + q0:(h2 + 1) * 512],
                            kT[b0:b0 + 64, hp, kt * 128:(kt + 1) * 128],
                            qT[b0:b0 + 64, hp, n * 512 + q0:(n + 1) * 512],
                            start=True,
                            stop=True,
                        )
                ex = work.tile([128, 2, 512], BF16, tag="ex", bufs=4)
                if "sc" in phases:
                    scv = sc[:].rearrange("p (h q) -> p h q", h=2)
                    nc.scalar.activation(
                        ex[:, :, q0:], scv[:, :, q0:],
                        mybir.ActivationFunctionType.Exp, scale=0.125)
                    if j >= 0:
                        nc.vector.tensor_mul(
                            ex[:, :, q0:q0 + 128], ex[:, :, q0:q0 + 128],
                            trit[:])
                if "av" in phases:
                    if pend is not None:
                        pend[0](*pend[1:])
                    pend = (emit_av, kt, q0, ex)
                if hp == 1 and kt == 1 and fin_pend:
                    fin_pend.pop(0)()
            if "av" in phases and pend is not None and hp == 1:
                pend[0](*pend[1:])
                pend = None
            fin_pend.append(make_fin(hp, ops))

    def wo_proj(qsb):
        oT = oT_hist.pop(qsb)
        for qb2 in range(4):
            qb = qsb * 4 + qb2
            yt = work.tile([128, E], BF16, tag="yt", bufs=3)
            for ec in range(2):
                yps = psum.tile([128, 512], F32, name="yps", tag="acc",
                                bufs=2)
                for hpt in range(2):
                    nc.tensor.matmul(
                        yps[:],
                        oT[hpt][:, qb2 * 128:(qb2 + 1) * 128],
                        Wor[:, hpt, ec * 512:(ec + 1) * 512],
                        start=(hpt == 0),
                        stop=(hpt == 1),
                    )
                nc.vector.tensor_copy(yt[:, ec * 512:(ec + 1) * 512], yps[:])
            eng = nc.sync if qb % 2 == 0 else nc.scalar
            eng.dma_start(yp[qb * 128:(qb + 1) * 128, :].bitcast(BF16),
                          yt[:])

    fin_pend = []
    for n in range(QSB):
        if "qkv" in phases:
            qkv_qk(n, fin_pend)
        else:
            while fin_pend:
                fin_pend.pop(0)()
        if "v" in phases:
            v_nat(n)
        if n >= 1 and "wo" in phases and (n - 1) in oT_hist:
            wo_proj(n - 1)
        if "sc" in phases or "av" in phases:
            attn(n, fin_pend)
    while fin_pend:
        fin_pend.pop(0)()
    if "wo" in phases and (QSB - 1) in oT_hist:
        wo_proj(QSB - 1)


def _build(repeat=1, loop=0, phases=ALL_PHASES):
    key = ("nc", repeat, loop, tuple(sorted(phases)))
    if key in _CACHE:
        return _CACHE[key]
    nc = bacc.Bacc("TRN2", target_bir_lowering=False, debug=False, num_devices=8)
    x = nc.dram_tensor("x", [E, S], mybir.dt.uint16, kind="ExternalInput").ap()
    wqkv = nc.dram_tensor("wqkv", [E, HC], mybir.dt.uint16,
                          kind="ExternalInput").ap()
    b2 = nc.dram_tensor("b2", [128, 6], F32, kind="ExternalInput").ap()
    wo = nc.dram_tensor("wo", [NH * HD, E], mybir.dt.uint16,
                        kind="ExternalInput").ap()
    vbias = nc.dram_tensor("vbias", [128, NH * HD], F32,
                           kind="ExternalInput").ap()
    tri = nc.dram_tensor("tri", [128, 256], mybir.dt.uint16,
                         kind="ExternalInput").ap()
    yp = nc.dram_tensor("yp", [S, E], mybir.dt.uint16,
                        kind="ExternalOutput").ap()
    with tile.TileContext(nc) as tc:
        if loop:
            with tc.For_i(0, loop, 1):
                _mha_kernel(tc, x, wqkv, b2, wo, vbias, tri, yp, phases=phases)
        else:
            for _ in range(repeat):
                _mha_kernel(tc, x, wqkv, b2, wo, vbias, tri, yp, phases=phases)
    nc.compile()
    _CACHE[key] = nc
    return nc


def _shard_inputs(x, Wqkv, bqkv, Wo, bo, mask):
    x = np.asarray(x, dtype=np.float32)
    Wqkv = np.asarray(Wqkv, dtype=np.float32)
    bqkv = np.asarray(bqkv, dtype=np.float32)
    Wo = np.asarray(Wo, dtype=np.float32)

    # inclusive lower-tri (k <= q) in [k, q] layout, duplicated for 2 heads
    tri1 = np.triu(np.ones((128, 128), dtype=np.float32))
    tri = np.concatenate([tri1, tri1], axis=1)
    tri_u16 = tri.astype(ml_dtypes.bfloat16).view(np.uint16)

    in_maps = []
    for c in range(8):
        b, g = divmod(c, 4)
        h0 = NH * g
        # columns of Wqkv for heads h0..h0+3, permuted so qkvT rows come out
        # grouped [q h0,h1 | q h2,h3 | k h0,h1 | k h2,h3 | v h0..h3]
        cols = []
        for t in range(3):
            for h in range(NH):
                base = 3 * HD * (h0 + h) + t * HD
                cols.extend(range(base, base + HD))
        cols = np.array(cols)
        b2 = bqkv[cols].reshape(6, 128).T.copy()          # [128, 6]
        vbias = np.ascontiguousarray(
            np.broadcast_to(bqkv[cols[512:768]], (128, NH * HD)))
        xt = np.ascontiguousarray(x[b].T).astype(ml_dtypes.bfloat16)
        wq = np.ascontiguousarray(Wqkv[:, cols]).astype(ml_dtypes.bfloat16)
        in_maps.append({
            "x": xt.view(np.uint16),
            "wqkv": wq.view(np.uint16),
            "b2": np.ascontiguousarray(b2),
            "wo": np.ascontiguousarray(
                Wo[HD * h0:HD * h0 + NH * HD, :]).astype(
                    ml_dtypes.bfloat16).view(np.uint16),
            "vbias": vbias,
            "tri": tri_u16,
        })
    return in_maps


def kernel(x, Wqkv, bqkv, Wo, bo, mask):
    global LAST_RESULT
    nc = _build()
    in_maps = _shard_inputs(x, Wqkv, bqkv, Wo, bo, mask)
    trace = bool(int(os.environ.get("KERNEL_TRACE", "0")))
    res = run_bass_kernel_spmd(nc, in_maps, list(range(8)), trace=trace)
    LAST_RESULT = res
    bo = np.asarray(bo, dtype=np.float32)
    y = np.empty((B, S, E), dtype=np.float32)
    for b in range(B):
        acc = res.results[4 * b]["yp"].view(ml_dtypes.bfloat16).astype(
            np.float32)
        for g in range(1, 4):
            acc = acc + res.results[4 * b + g]["yp"].view(
                ml_dtypes.bfloat16).astype(np.float32)
        y[b] = acc + bo[None, :]
    return y


# revision 20
# speedup vs baseline: 1.0810x; 1.0810x over previous
"""Multi-head attention (B=2, S=2048, E=1024, H=16, hd=64) on 8 trn2 cores.

Sharding: core c handles batch b = c//4 and 4 heads h0 = 4*(c%4).
Each core computes its heads' attention output projected through its rows
of Wo (tensor-parallel row split); the host sums the 4 partials per batch
and adds bo.

Device-side dataflow (per core, feature-major "transposed" layouts):
  xT   [e,s]  bf16  <- host-pretransposed x[b]
  qT,kT[d,s]  f32   <- Wqkv_slice^T @ xT   (bf16 matmul, fp32 psum)
  v1   [s,d]  bf16  <- xT-tile-stationary matmul (natural v, no transpose)
  scT  [k,2*q] f32  <- kT^T @ qT, both heads of a pair in one psum tile
  ex   [k,2*q] bf16 <- exp(scT/8) (one Act op per k-tile), diag tri mask
  ops  [65,2*q] f32 <- [v|1]^T @ ex  (bf16 matmul, fused denominator row)
  oT   [d,q]  f32r  <- ops * (1/den) (Pool partition_broadcast + DVE/Pool mul)
  y    [q,e]  bf16  <- oT^T @ Wo_slice (f32r matmul), psum->sbuf->HBM
"""

import os
import sys

sys.path.insert(0, "/opt/trn_rl_repo")

from contextlib import ExitStack

import ml_dtypes
import numpy as np

import concourse.bass as bass
import concourse.tile as tile
from concourse import bacc, mybir
from concourse._compat import with_exitstack
from concourse.bass_utils import run_bass_kernel_spmd

B, S, E, H = 2, 2048, 1024, 16
HD = E // H            # 64
NH = 4                 # heads per core
HC = NH * 3 * HD       # 768 qkv columns per core
ET = E // 128          # 8 e-tiles
QSB = S // 512         # 4 query super-blocks
KT = S // 128          # 16 k tiles
VP = 80                # v1 per-head stride: 64 v cols + 1 ones col + pad
F32 = mybir.dt.float32
F32R = mybir.dt.float32r
BF16 = mybir.dt.bfloat16

_CACHE = {}
LAST_RESULT = None

ALL_PHASES = frozenset({"x", "qkv", "v", "sc", "av", "wo"})


@with_exitstack
def _mha_kernel(ctx: ExitStack, tc: tile.TileContext, x, wqkv, b2, wo, vbias,
                tri, yp, phases=ALL_PHASES):
    nc = tc.nc

    const = ctx.enter_context(tc.tile_pool(name="const", bufs=1))
    work = ctx.enter_context(tc.tile_pool(name="work", bufs=1))
    psum = ctx.enter_context(tc.tile_pool(name="psum", bufs=1, space="PSUM"))

    # ---- persistent SBUF tensors ----
    W = const.tile([128, ET, HC], BF16)          # Wqkv slice, e-major
    Wor = const.tile([128, 2, E], BF16)          # Wo slice rows (hd), bf16
    bqk = const.tile([128, 6], F32)              # q/k bias, d-major per m-tile
    vb_t = const.tile([128, NH * HD], F32)       # v bias broadcast over rows
    trit = const.tile([128, 2, 128], BF16)       # causal tri mask, both heads
    xT = const.tile([128, ET, S], BF16)          # x[b] transposed, bf16
    qT = const.tile([128, 2, S], BF16)           # per head-pair: even@0:64 rows
    kT = const.tile([128, 2, S], BF16)
    v1 = const.tile([128, KT, NH * VP], BF16)    # v natural + ones col per head

    # ---- const loads (sync queue; small tensors first, they gate compute)
    nc.sync.dma_start(bqk[:], b2[:, :])
    nc.sync.dma_start(vb_t[:], vbias[:, :])
    nc.sync.dma_start(trit[:], tri[:, :].bitcast(BF16).rearrange(
        "p (h q) -> p h q", h=2))
    # W as 3 column-slab DMAs (q|k|v), one issue each, 16-engine parallel
    for t in range(ET):
        nc.sync.dma_start(W[:, t, :],
                          wqkv[t * 128:(t + 1) * 128, :].bitcast(BF16))
    # x as 4 s-block slabs: qkv(n) gated only on slab n
    for half in range(2) if "x" in phases else []:
        for t in range(ET):
            nc.sync.dma_start(
                xT[:, t, half * 1024:(half + 1) * 1024],
                x[t * 128:(t + 1) * 128,
                  half * 1024:(half + 1) * 1024].bitcast(BF16))
    for t in range(2):
        nc.sync.dma_start(Wor[:, t, :],
                          wo[t * 128:(t + 1) * 128, :].bitcast(BF16))
    for h in range(NH):
        nc.vector.memset(v1[:, :, h * VP + HD:h * VP + HD + 1], 1.0)
    ones64 = const.tile([1, 64], BF16, name="ones64")
    nc.vector.memset(ones64[:], 1.0)
    # warm the Act table (Exp) while DMAs stream, off the critical path
    warm = work.tile([1, 64], F32, name="warm", tag="warm", bufs=1)
    nc.scalar.activation(warm[:], ones64[:],
                         mybir.ActivationFunctionType.Exp, scale=0.125)

    oT_hist = {}

    def qkv_qk(n, fin_pend):
        # qT/kT for s-block n: W-tile stationary, xT moving
        for m in range(4):
            ps = psum.tile([128, 512], F32, tag="acc", bufs=2)
            for et in range(ET):
                nc.tensor.matmul(
                    ps[:],
                    W[:, et, m * 128:(m + 1) * 128],
                    xT[:, et, n * 512:(n + 1) * 512],
                    start=(et == 0),
                    stop=(et == ET - 1),
                )
            typ, hp = m // 2, m % 2
            dst = (qT if typ == 0 else kT)[:, hp, n * 512:(n + 1) * 512]
            if typ == 0:
                nc.vector.tensor_scalar_add(dst, ps[:], bqk[:, m:m + 1])
            else:
                nc.scalar.activation(
                    dst, ps[:], mybir.ActivationFunctionType.Identity,
                    bias=bqk[:, m:m + 1])
            if m == 0 and fin_pend:
                fin_pend.pop(0)()

    def v_nat(n):
        # natural-layout v via xT-tile-stationary matmul (no PE transpose)
        for st in range(4 * n, 4 * n + 4):
            vp = psum.tile([128, NH * HD], F32, name="vp", tag="big", bufs=2)
            for et in range(ET):
                nc.tensor.matmul(
                    vp[:],
                    xT[:, et, st * 128:(st + 1) * 128],
                    W[:, et, 512:768],
                    start=(et == 0),
                    stop=(et == ET - 1),
                )
            dst = v1[:, st, :].rearrange("p (h c) -> p h c", h=NH)[:, :, :HD]
            nc.vector.tensor_add(
                dst,
                vp[:].rearrange("p (h c) -> p h c", h=NH),
                vb_t[:].rearrange("p (h c) -> p h c", h=NH),
            )

    def attn(n, fin_pend):
        nkt = 4 * (n + 1)
        oT = [
            work.tile([128, 512], BF16, name=f"oT{hp}", tag=f"oT{hp}", bufs=2)
            for hp in range(2)
        ]
        oT_hist[n] = oT

        def make_fin(hp, ops):
            def fin():
                rc = work.tile([1, 1024], BF16, tag="rc", bufs=2)
                with nc.allow_low_precision(reason="den fits bf16"):
                    nc.vector.reciprocal(rc[:], ops[64:65, :])
                rb = psum.tile([64, 1024], F32, name="rb", tag="big", bufs=2)
                for h2 in range(2):
                    nc.tensor.matmul(
                        rb[:, h2 * 512:(h2 + 1) * 512],
                        ones64[:],
                        rc[:, h2 * 512:(h2 + 1) * 512],
                        start=True, stop=True)
                rbs = work.tile([64, 1024], F32, tag="rbs", bufs=2)
                nc.vector.tensor_copy(rbs[:], rb[:])
                for h2 in range(2):
                    nc.vector.tensor_mul(
                        oT[hp][64 * h2:64 * h2 + 64, :],
                        ops[0:64, h2 * 512:(h2 + 1) * 512],
                        rbs[0:64, h2 * 512:(h2 + 1) * 512])
            return fin

        pend = None
        for hp in range(2):
            ops = psum.tile([65, 1024], F32, name="ops", tag="acc", bufs=2)

            def emit_av(kt, q0, ex, ops=ops, hp=hp):
                for h2 in range(2):
                    h = 2 * hp + h2
                    nc.tensor.matmul(
                        ops[:, h2 * 512 + q0:(h2 + 1) * 512],
                        v1[:, kt, h * VP:h * VP + HD + 1],
                        ex[:, h2, q0:],
                        start=(kt == 0),
                        stop=(kt == nkt - 1),
                    )

            for kt in range(nkt):
                j = kt - 4 * n
                q0 = max(0, j * 128)
                sc = psum.tile([128, 1024], F32, tag="big", bufs=2)
                if "sc" in phases:
                    for h2 in range(2):
                        b0 = 64 * h2
                        nc.tensor.matmul(
                            sc[:, h2 * 512 + q0:(h2 + 1) * 512],
                            kT[b0:b0 + 64, hp, kt * 128:(kt + 1) * 128],
                            qT[b0:b0 + 64, hp, n * 512 + q0:(n + 1) * 512],
                            start=True,
                            stop=True,
                        )
                ex = work.tile([128, 2, 512], BF16, tag="ex", bufs=4)
                if "sc" in phases:
                    scv = sc[:].rearrange("p (h q) -> p h q", h=2)
                    nc.scalar.activation(
                        ex[:, :, q0:], scv[:, :, q0:],
                        mybir.ActivationFunctionType.Exp, scale=0.125)
                    if j >= 0:
                        nc.vector.tensor_mul(
                            ex[:, :, q0:q0 + 128], ex[:, :, q0:q0 + 128],
                            trit[:])
                if "av" in phases:
                    if pend is not None:
                        pend[0](*pend[1:])
                    pend = (emit_av, kt, q0, ex)
                if hp == 1 and kt == 1 and fin_pend:
                    fin_pend.pop(0)()
            if "av" in phases and pend is not None and hp == 1:
                pend[0](*pend[1:])
                pend = None
            fin_pend.append(make_fin(hp, ops))

    def wo_proj(qsb):
        oT = oT_hist.pop(qsb)
        for qb2 in range(4):
            qb = qsb * 4 + qb2
            yt = work.tile([128, E], BF16, tag="yt", bufs=3)
            for ec in range(2):
                yps = psum.tile([128, 512], F32, name="yps", tag="acc",
                                bufs=2)
                for hpt in range(2):
                    nc.tensor.matmul(
                        yps[:],
                        oT[hpt][:, qb2 * 128:(qb2 + 1) * 128],
                        Wor[:, hpt, ec * 512:(ec + 1) * 512],
                        start=(hpt == 0),
                        stop=(hpt == 1),
                    )
                nc.vector.tensor_copy(yt[:, ec * 512:(ec + 1) * 512], yps[:])
            eng = nc.sync if qb % 2 == 0 else nc.scalar
            eng.dma_start(yp[qb * 128:(qb + 1) * 128, :].bitcast(BF16),
                          yt[:])

    fin_pend = []
    for n in range(QSB):
        if "qkv" in phases:
            qkv_qk(n, fin_pend)
        else:
            while fin_pend:
                fin_pend.pop(0)()
        if "v" in phases:
            v_nat(n)
        if n >= 1 and "wo" in phases and (n - 1) in oT_hist:
            wo_proj(n - 1)
        if "sc" in phases or "av" in phases:
            attn(n, fin_pend)
    while fin_pend:
        fin_pend.pop(0)()
    if "wo" in phases and (QSB - 1) in oT_hist:
        wo_proj(QSB - 1)


def _build(repeat=1, loop=0, phases=ALL_PHASES):
    key = ("nc", repeat, loop, tuple(sorted(phases)))
    if key in _CACHE:
        return _CACHE[key]
    nc = bacc.Bacc("TRN2", target_bir_lowering=False, debug=False, num_devices=8)
    x = nc.dram_tensor("x", [E, S], mybir.dt.uint16, kind="ExternalInput").ap()
    wqkv = nc.dram_tensor("wqkv", [E, HC], mybir.dt.uint16,
                          kind="ExternalInput").ap()
    b2 = nc.dram_tensor("b2", [128, 6], F32, kind="ExternalInput").ap()
    wo = nc.dram_tensor("wo", [NH * HD, E], mybir.dt.uint16,
                        kind="ExternalInput").ap()
    vbias = nc.dram_tensor("vbias", [128, NH * HD], F32,
                           kind="ExternalInput").ap()
    tri = nc.dram_tensor("tri", [128, 256], mybir.dt.uint16,
                         kind="ExternalInput").ap()
    yp = nc.dram_tensor("yp", [S, E], mybir.dt.uint16,
                        kind="ExternalOutput").ap()
    with tile.TileContext(nc) as tc:
        if loop:
            with tc.For_i(0, loop, 1):
                _mha_kernel(tc, x, wqkv, b2, wo, vbias, tri, yp, phases=phases)
        else:
            for _ in range(repeat):
                _mha_kernel(tc, x, wqkv, b2, wo, vbias, tri, yp, phases=phases)
    nc.compile()
    _CACHE[key] = nc
    return nc


def _shard_inputs(x, Wqkv, bqkv, Wo, bo, mask):
    x = np.asarray(x, dtype=np.float32)
    Wqkv = np.asarray(Wqkv, dtype=np.float32)
    bqkv = np.asarray(bqkv, dtype=np.float32)
    Wo = np.asarray(Wo, dtype=np.float32)

    # inclusive lower-tri (k <= q) in [k, q] layout, duplicated for 2 heads
    tri1 = np.triu(np.ones((128, 128), dtype=np.float32))
    tri = np.concatenate([tri1, tri1], axis=1)
    tri_u16 = tri.astype(ml_dtypes.bfloat16).view(np.uint16)

    in_maps = []
    for c in range(8):
        b, g = divmod(c, 4)
        h0 = NH * g
        # columns of Wqkv for heads h0..h0+3, permuted so qkvT rows come out
        # grouped [q h0,h1 | q h2,h3 | k h0,h1 | k h2,h3 | v h0..h3]
        cols = []
        for t in range(3):
            for h in range(NH):
                base = 3 * HD * (h0 + h) + t * HD
                cols.extend(range(base, base + HD))
        cols = np.array(cols)
        b2 = bqkv[cols].reshape(6, 128).T.copy()          # [128, 6]
        vbias = np.ascontiguousarray(
            np.broadcast_to(bqkv[cols[512:768]], (128, NH * HD)))
        xt = np.ascontiguousarray(x[b].T).astype(ml_dtypes.bfloat16)
        wq = np.ascontiguousarray(Wqkv[:, cols]).astype(ml_dtypes.bfloat16)
        in_maps.append({
            "x": xt.view(np.uint16),
            "wqkv": wq.view(np.uint16),
            "b2": np.ascontiguousarray(b2),
            "wo": np.ascontiguousarray(
                Wo[HD * h0:HD * h0 + NH * HD, :]).astype(
                    ml_dtypes.bfloat16).view(np.uint16),
            "vbias": vbias,
            "tri": tri_u16,
        })
    return in_maps


def kernel(x, Wqkv, bqkv, Wo, bo, mask):
    global LAST_RESULT
    nc = _build()
    in_maps = _shard_inputs(x, Wqkv, bqkv, Wo, bo, mask)
    trace = bool(int(os.environ.get("KERNEL_TRACE", "0")))
    res = run_bass_kernel_spmd(nc, in_maps, list(range(8)), trace=trace)
    LAST_RESULT = res
    bo = np.asarray(bo, dtype=np.float32)
    y = np.empty((B, S, E), dtype=np.float32)
    for b in range(B):
        acc = res.results[4 * b]["yp"].view(ml_dtypes.bfloat16).astype(
            np.float32)
        for g in range(1, 4):
            acc = acc + res.results[4 * b + g]["yp"].view(
                ml_dtypes.bfloat16).astype(np.float32)
        y[b] = acc + bo[None, :]
    return y
